# revision 1
# baseline (speedup 1.0000x reference)
"""Trainium2 Bass kernel for nn_Metamorph_parameterReinforcer.

Math background (exact identities, verified against the reference):
  The reference's einsum("bfp,mn->bfm", fx, wfft) sums over BOTH p and n,
  so each "STFT block" collapses:
    sum_p fft(x, norm=forward)[..., p] == x[..., 0]
    block(x)[b, f, k] = Re tanh(x[b, f, 0] * W[k]),
       W[k] = sum_m (sum_n wfft[m, n]) * exp(2j*pi*k*m/64)
  Chaining three blocks, only element 0 of the last axis propagates:
    a  = params[:, :, 0]
    s1 = Retanh(a  * W0[0]);  s2 = Retanh(s1 * W1[0])
    x3[b, f, l] = Retanh(s2[b, f] * W2[l])         # (512, 1000, 64)
    h  = tanh(x3.reshape(512, 64000) @ lin1_w.T + lin1_b)
    out = sigmoid(h @ lin2_w.T + lin2_b)
  Because |W0[0]|, |W1[0]| ~ 32000 (sums of 64000 uniforms), tanh saturates
  and s2 is exactly +-1 in f32 for all but (rare) |a| < ~1e-4 entries. Where
  s2 is exactly +-1, x3[b, f, :] = s2[b, f] * X1[:] with X1 = Retanh(W2) --
  exactly rank-1. Rare non-saturated entries are handled by an exact
  correction term dH added before the lin1 tanh (computed on host from the
  few affected (b, f) pairs; zero for typical inputs).

Device kernel (8 cores, lin1_w sharded over its output dim j, 125 rows/core;
the 256 MB lin1_w read is the memory roofline and is read exactly once
across the fleet):
  stage 1: A[j, f] = sum_l X1[l] * w1[j, 64 f + l]      (TensorE)
           K-packs two f per matmul: lhsT = w1 tile [(f', l)=128, j=125],
           rhs = block-diag X1 [(f', l)=128, 2] -> out [j=125, 2] per pair.
  stage 2: A -> A_T via PE transpose; h[j, b] = tanh(sum_f A_T[f, j] *
           s2T[f, b] + lin1_b[j] (+ dH)) -- K=f matmuls + ScalarE tanh.
  stage 3: partial[k, b] = sum_j lin2_w[k, j] * h[j, b]  (one matmul)
Host combines the 8 partials: out = sigmoid(sum_c partial_c + lin2_b).
"""

import numpy as np

B, MODES, L = 512, 1000, 64
NCORES = 8
JSH = MODES // NCORES          # 125 lin1 output rows per core
NGRP = MODES // 4              # 250 four-f groups for the M4 stage 1
NTOT = NGRP * JSH              # 31250 stage-1 outputs (g, j) per core
NCH = 4 * JSH                  # psum chunk: 4 g x 125 j = 500 columns
BIGCH = 5 * NCH                # DMA chunk (2500 cols x 2 halves, 1.25 MB)
SAT = 50.0                     # |2*s*Re(W)| beyond this: Retanh == sign
SAT = 50.0                     # |2*s*Re(W)| beyond this: Retanh == sign


def _retanh(s, w):
    """Re tanh(s * w) for real array s and complex (array or scalar) w."""
    s = np.asarray(s, np.float64)
    x = 2.0 * np.multiply.outer(s, np.real(w))
    y = 2.0 * np.multiply.outer(s, np.imag(w))
    xc = np.clip(x, -SAT, SAT)
    with np.errstate(over="ignore", invalid="ignore"):
        r = np.sinh(xc) / (np.cosh(xc) + np.cos(y))
    return np.where(np.abs(x) >= SAT, np.sign(x), r)


def _wvec(wre, wim):
    """W[k] = sum_m (sum_n w[m, n]) * exp(2j pi k m / L)."""
    wsum = wre.astype(np.float64).sum(axis=1) + 1j * wim.astype(np.float64).sum(axis=1)
    tw = np.exp(2j * np.pi * np.outer(np.arange(L), np.arange(L)) / L)
    return tw @ wsum


_CACHE = {}


def _build_program(use_dh):
    """Build (and cache) the Bass program. Same program for all 8 cores."""
    key = ("prog", use_dh, "m4v3", NCH, BIGCH)
    if key in _CACHE:
        return _CACHE[key]

    import concourse.bacc as bacc
    import concourse.mybir as mybir
    import concourse.tile as tile

    f32 = mybir.dt.float32
    bf16 = mybir.dt.bfloat16
    nc = bacc.Bacc("TRN2", target_bir_lowering=False, debug=False)

    w1x_d = nc.dram_tensor("w1x", [128, 2, NTOT], bf16, kind="ExternalInput")
    s2t_d = nc.dram_tensor("s2t", [MODES, B], bf16, kind="ExternalInput")
    x1d4_d = nc.dram_tensor("x1d4", [128, 8], bf16, kind="ExternalInput")
    bias_d = nc.dram_tensor("bias", [JSH, 1], f32, kind="ExternalInput")
    l2t_d = nc.dram_tensor("l2t", [JSH, L], f32, kind="ExternalInput")
    if use_dh:
        dht_d = nc.dram_tensor("dht", [JSH, B], f32, kind="ExternalInput")
    outp_d = nc.dram_tensor("outp", [L, B], f32, kind="ExternalOutput")

    n_ft = (MODES + 127) // 128          # 8 f-tiles for stage 2

    with tile.TileContext(nc) as tc:
        with (
            tc.tile_pool(name="const", bufs=1) as const,
            tc.tile_pool(name="w1pool", bufs=5) as w1pool,
            tc.tile_pool(name="acc", bufs=1) as acc,
            tc.tile_pool(name="psC", bufs=3, space="PSUM") as psC,
            tc.tile_pool(name="psH", bufs=1, space="PSUM") as psH,
            tc.tile_pool(name="psO", bufs=1, space="PSUM") as psO,
        ):
            x1d4 = const.tile([128, 8], bf16)
            nc.sync.dma_start(x1d4[:], x1d4_d.ap())
            bias = const.tile([JSH, 1], f32)
            nc.sync.dma_start(bias[:], bias_d.ap())
            l2t = const.tile([JSH, L], f32)
            nc.sync.dma_start(l2t[:], l2t_d.ap())
            s2t = const.tile([128, n_ft * B], bf16)
            for t in range(n_ft):
                ft = min(128, MODES - 128 * t)
                nc.scalar.dma_start(
                    s2t[0:ft, B * t : B * (t + 1)],
                    s2t_d.ap()[128 * t : 128 * t + ft, :],
                )
            if use_dh:
                dht = const.tile([JSH, B], f32)
                nc.sync.dma_start(dht[:], dht_d.ap())

            # ---- stage 1 (TensorE): S[fp, g, j] = sum_l X1[l] w1[j, 4g+fp, l]
            # lhsT = block-diag X1 halves [K=(fp,lh)=128, 4]; rhs = w1x
            # chunks [128, 500]; two matmuls (l low/high) accumulate in PSUM.
            s4 = acc.tile([4, NGRP, JSH], bf16)
            dma_engines = [nc.sync, nc.scalar]
            ev = 0
            n_big = (NTOT + BIGCH - 1) // BIGCH
            for bc in range(n_big):
                n0 = bc * BIGCH
                nn_big = min(BIGCH, NTOT - n0)
                w1c = w1pool.tile([128, 2, BIGCH], bf16, tag="w1c")
                dma_engines[bc % 2].dma_start(
                    w1c[:, :, 0:nn_big], w1x_d.ap()[:, :, n0 : n0 + nn_big]
                )
                for off in range(0, nn_big, NCH):
                    nn = min(NCH, nn_big - off)
                    gn = nn // JSH
                    g0 = (n0 + off) // JSH
                    pc = psC.tile([4, NCH], f32, tag="pc")
                    nc.tensor.matmul(
                        pc[0:4, 0:nn],
                        x1d4[:, 0:4],
                        w1c[:, 0, off : off + nn],
                        start=True,
                        stop=False,
                    )
                    nc.tensor.matmul(
                        pc[0:4, 0:nn],
                        x1d4[:, 4:8],
                        w1c[:, 1, off : off + nn],
                        start=False,
                        stop=True,
                    )
                    src = pc[0:4, 0:nn].rearrange("p (g j) -> p g j", j=JSH)
                    dst = s4[0:4, g0 : g0 + gn, :]
                    if ev % 2 == 0:
                        nc.vector.tensor_copy(dst, src)
                    else:
                        nc.scalar.activation(
                            dst, src, mybir.ActivationFunctionType.Copy
                        )
                    ev += 1

            # ---- scatter S[fp, g, j] -> A_T[fhat = 250 fp + g, j] ----
            # (stage 2 contracts over fhat; s2t rows are host-permuted to match)
            at_sb = acc.tile([128, n_ft * JSH], bf16)
            for fp in range(4):
                a = 250 * fp
                end = 250 * (fp + 1)
                while a < end:
                    t = a // 128
                    b_ = min(end, 128 * (t + 1))
                    p0 = a - 128 * t
                    ln = b_ - a
                    g0 = a - 250 * fp
                    nc.gpsimd.dma_start(
                        at_sb[p0 : p0 + ln, JSH * t : JSH * (t + 1)],
                        s4[fp : fp + 1, g0 : g0 + ln, :],
                    )
                    a = b_

            # ---- stage 2: h[j, b] = tanh(sum_f A_T[f, j] s2t[f, b] + bias) ----
            ph = psH.tile([JSH, B], f32)
            for t in range(n_ft):
                ft = min(128, MODES - 128 * t)
                nc.tensor.matmul(
                    ph[:, :],
                    at_sb[0:ft, JSH * t : JSH * (t + 1)],
                    s2t[0:ft, B * t : B * (t + 1)],
                    start=(t == 0),
                    stop=(t == n_ft - 1),
                )
            if use_dh:
                nc.vector.tensor_add(ph[:, :], ph[:, :], dht[:, :])
            h_sb = acc.tile([JSH, B], f32)
            nc.scalar.activation(
                h_sb[:, :],
                ph[:, :],
                mybir.ActivationFunctionType.Tanh,
                bias=bias[:, 0:1],
            )

            # ---- stage 3: partial[k, b] = sum_j l2t[j, k] h[j, b] ----
            po = psO.tile([L, B], f32)
            nc.tensor.matmul(po[:, :], l2t[:, :], h_sb[:, :], start=True, stop=True)
            o_sb = acc.tile([L, B], f32)
            nc.vector.tensor_copy(o_sb[:, :], po[:, :])
            nc.sync.dma_start(outp_d.ap(), o_sb[:, :])

    nc.compile()
    _CACHE[key] = nc
    return nc


def profile_last(trace_cores=None):
    """Re-run the last-built program with NTFF tracing (dev/test helper)."""
    if "last_run" not in _CACHE:
        return None
    from concourse.bass_utils import run_bass_kernel_spmd

    nc, in_maps = _CACHE["last_run"]
    return run_bass_kernel_spmd(
        nc,
        in_maps,
        list(range(NCORES)),
        trace=True,
        trace_cores=trace_cores,
    )


def kernel(
    params,
    wfft0_re,
    wfft0_im,
    wfft1_re,
    wfft1_im,
    wfft2_re,
    wfft2_im,
    lin1_w,
    lin1_b,
    lin2_w,
    lin2_b,
):
    from concourse.bass_utils import run_bass_kernel_spmd

    # ---- host: closed-form collapse of the three spectral blocks ----
    a = params[:, :, 0].astype(np.float64)
    w0 = _wvec(wfft0_re, wfft0_im)[0]
    w1v = _wvec(wfft1_re, wfft1_im)[0]
    w2 = _wvec(wfft2_re, wfft2_im)
    s1 = _retanh(a, w0)
    s2 = _retanh(s1, w1v).astype(np.float32)
    x1 = _retanh(np.float64(1.0), w2).astype(np.float32)  # (64,)

    # exact correction for entries where tanh did not saturate to +-1
    bad_b, bad_f = np.nonzero(np.abs(s2) != np.float32(1.0))
    use_dh = bad_b.size > 0
    dh = None
    if use_dh:
        dh = np.zeros((B, MODES), np.float64)
        x1_64 = x1.astype(np.float64)
        for b, f in zip(bad_b.tolist(), bad_f.tolist()):
            s = np.float64(s2[b, f])
            delta = _retanh(s, w2)[0] - s * x1_64
            dh[b, :] += lin1_w[:, 64 * f : 64 * (f + 1)].astype(np.float64) @ delta
        dh = dh.astype(np.float32)

    # ---- host: per-core shards / layouts ----
    import ml_dtypes

    bf16 = ml_dtypes.bfloat16
    # stage-2 contraction order fhat = 250*fp + g  <->  f = 4*g + fp
    fhat = np.arange(MODES)
    perm = 4 * (fhat % 250) + fhat // 250
    s2t = np.ascontiguousarray(s2.T[perm].astype(bf16))   # (1000, 512)

    x1d4 = np.zeros((128, 8), np.float32)
    for h in range(2):
        for fp in range(4):
            x1d4[fp * 32 : (fp + 1) * 32, 4 * h + fp] = x1[32 * h : 32 * (h + 1)]
    x1d4 = x1d4.astype(bf16)

    in_maps = []
    for c in range(NCORES):
        j0, j1 = JSH * c, JSH * (c + 1)
        # w1x[half, (fp, lh), (g, j)] = w1[j, 64*(4g+fp) + 32*half + lh]
        # [K=(fp,lh)=128, half, N=(g,j)]
        w1x = np.ascontiguousarray(
            lin1_w[j0:j1]
            .reshape(JSH, NGRP, 4, 2, 32)
            .transpose(2, 4, 3, 1, 0)
            .reshape(128, 2, NTOT)
            .astype(bf16)
        )
        m = {
            "w1x": w1x,
            "s2t": s2t,
            "x1d4": x1d4,
            "bias": np.ascontiguousarray(lin1_b[j0:j1].reshape(JSH, 1)),
            "l2t": np.ascontiguousarray(lin2_w[:, j0:j1].T),
        }
        if use_dh:
            m["dht"] = np.ascontiguousarray(dh[:, j0:j1].T)
        in_maps.append(m)

    nc = _build_program(use_dh)
    _CACHE["last_run"] = (nc, in_maps)
    res = run_bass_kernel_spmd(nc, in_maps, list(range(NCORES)))

    acc = np.zeros((L, B), np.float64)
    for c in range(NCORES):
        acc += res.results[c]["outp"].astype(np.float64)
    out = 1.0 / (1.0 + np.exp(-(acc.T + lin2_b.astype(np.float64))))
    return out.astype(np.float32)



# revision 5
# speedup vs baseline: 1.6926x; 1.6926x over previous
"""Trainium2 Bass kernel for nn_Metamorph_parameterReinforcer.

Math background (exact identities, verified against the reference):
  The reference's einsum("bfp,mn->bfm", fx, wfft) sums over BOTH p and n,
  so each "STFT block" collapses:
    sum_p fft(x, norm=forward)[..., p] == x[..., 0]
    block(x)[b, f, k] = Re tanh(x[b, f, 0] * W[k]),
       W[k] = sum_m (sum_n wfft[m, n]) * exp(2j*pi*k*m/64)
  Chaining three blocks, only element 0 of the last axis propagates:
    a  = params[:, :, 0]
    s1 = Retanh(a  * W0[0]);  s2 = Retanh(s1 * W1[0])
    x3[b, f, l] = Retanh(s2[b, f] * W2[l])         # (512, 1000, 64)
    h  = tanh(x3.reshape(512, 64000) @ lin1_w.T + lin1_b)
    out = sigmoid(h @ lin2_w.T + lin2_b)
  Because |W0[0]|, |W1[0]| ~ 32000, tanh saturates and s2 is +-1 in f32 for
  all but (rare) tiny-|a| entries; where s2 = +-1, x3[b, f, :] =
  s2[b, f] * X1[:] with X1 = Retanh(W2) -- exactly rank-1.  Rare
  non-saturated entries get an exact correction (dht) added before the lin1
  tanh.

Device kernel (8 cores, lin1_w sharded over its output dim j, 125 rows/core).
v2 design (fp8 + DoubleRow + constant stationary operand):
  * X1 is folded into lin1_w on the host: w1'[o, f, l] = lin1_w[o, 64f+l] *
    X1[l].  Stage 1 then reduces w1' over l with a CONSTANT ones
    block-diagonal stationary operand, so every matmul shares one lhsT and
    columns from different f-groups ride in the same matmul.
  * w1' is quantized to fp8e4 (TRN E4M3, max 240) with ERROR-FEEDBACK along
    l: the rounding error of element l is carried into element l+1, so the
    per-(o,f) SUM over l is accurate to ~1 ulp instead of sqrt(64) ulps.
    Simulated end-to-end rel err 2.4e-3 (vs 1.7e-2 naive fp8, 1.5e-3 bf16).
  * DoubleRow perf mode: virtual K=256 (2 fp8 per PE cell), halving both
    DMA bytes (8.4 MB/core) and PE streaming time vs bf16.  DoubleRow
    outputs must sit at psum partition 0 (walrus ISA check), so:
  * f padded 1000->1024, split into 8 banks of 128.  Bank B accumulates
    psum [32, 500] over 8 passes: psum[m, 125*c4 + j] = A[f, j] with
    f = 128B + 32*c4 + m.  One chunk = one bank's full fp8 stream (1 MB).
  * Per bank: DVE/ScalarE copy [32, 500] psum->SBUF (bf16), then ONE
    gpsimd SBUF->SBUF DMA reshapes [32, 500] -> [128, 125] (partition p =
    4m + c4; the host permutes s2t rows to match), giving the [128 f, 125 j]
    lhsT tile for stage 2.  All per-bank work overlaps later banks' stream.
  * stage 2: h[j, b] = tanh(2^-11 * sum_f A[f, j] s2[f, b] + bias) -- 8
    accumulating matmuls (K = 128 f's each) + ScalarE tanh (the activation
    scale folds the fp8 2^11 quantization scale away).
  * stage 3: partial[k, b] = sum_j l2t[j, k] h[j, b]  (one matmul)
Host combines the 8 partials: out = sigmoid(sum_c partial_c + lin2_b).
"""

import numpy as np

B, MODES, L = 512, 1000, 64
NCORES = 8
JSH = MODES // NCORES          # 125 lin1 output rows per core
FPAD = 1024                    # padded mode count (8 stage-2 K-tiles of 128)
NCHUNK = 8                     # w1x DMA chunks = psum banks (1 MB each)
CHCOL = 4000                   # columns per chunk: (P, c4, j) = 8*4*125
SCALE = 2.0 ** 11              # fp8 quantization scale for w1'
SAT = 50.0                     # |2*s*Re(W)| beyond this: Retanh == sign


def _retanh(s, w):
    """Re tanh(s * w) for real array s and complex (array or scalar) w."""
    s = np.asarray(s, np.float64)
    x = 2.0 * np.multiply.outer(s, np.real(w))
    y = 2.0 * np.multiply.outer(s, np.imag(w))
    xc = np.clip(x, -SAT, SAT)
    with np.errstate(over="ignore", invalid="ignore"):
        r = np.sinh(xc) / (np.cosh(xc) + np.cos(y))
    return np.where(np.abs(x) >= SAT, np.sign(x), r)


def _wvec(wre, wim):
    """W[k] = sum_m (sum_n w[m, n]) * exp(2j pi k m / L)."""
    wsum = wre.astype(np.float64).sum(axis=1) + 1j * wim.astype(np.float64).sum(axis=1)
    tw = np.exp(2j * np.pi * np.outer(np.arange(L), np.arange(L)) / L)
    return tw @ wsum


_CACHE = {}


def _build_program(use_dh):
    """Build (and cache) the Bass program. Same program for all 8 cores."""
    key = ("prog", use_dh, "fp8dr_v2")
    if key in _CACHE:
        return _CACHE[key]

    import concourse.bacc as bacc
    import concourse.mybir as mybir
    import concourse.tile as tile

    f32 = mybir.dt.float32
    bf16 = mybir.dt.bfloat16
    fp8 = mybir.dt.float8e4
    DR = mybir.MatmulPerfMode.DoubleRow
    nc = bacc.Bacc("TRN2", target_bir_lowering=False, debug=False)

    w1x_d = nc.dram_tensor("w1x", [128, 2, NCHUNK * CHCOL], fp8, kind="ExternalInput")
    s2t_d = nc.dram_tensor("s2t", [128, 8 * B], bf16, kind="ExternalInput")
    ones_d = nc.dram_tensor("ones8", [128, 2, 32], fp8, kind="ExternalInput")
    bias_d = nc.dram_tensor("bias", [JSH, 1], f32, kind="ExternalInput")
    l2t_d = nc.dram_tensor("l2t", [JSH, L], f32, kind="ExternalInput")
    if use_dh:
        dht_d = nc.dram_tensor("dht", [JSH, B], f32, kind="ExternalInput")
    outp_d = nc.dram_tensor("outp", [L, B], f32, kind="ExternalOutput")

    with tile.TileContext(nc) as tc:
        with (
            tc.tile_pool(name="const", bufs=1) as const,
            tc.tile_pool(name="w1pool", bufs=3) as w1pool,
            tc.tile_pool(name="s4pool", bufs=2) as s4pool,
            tc.tile_pool(name="acc", bufs=1) as acc,
            tc.tile_pool(name="psA", bufs=3, space="PSUM") as psA,
            tc.tile_pool(name="psH", bufs=1, space="PSUM") as psH,
            tc.tile_pool(name="psO", bufs=1, space="PSUM") as psO,
        ):
            ones8 = const.tile([128, 2, 32], fp8)
            nc.sync.dma_start(ones8[:], ones_d.ap())
            bias = const.tile([JSH, 1], f32)
            nc.sync.dma_start(bias[:], bias_d.ap())
            l2t = const.tile([JSH, L], f32)
            nc.sync.dma_start(l2t[:], l2t_d.ap())
            if use_dh:
                dht = const.tile([JSH, B], f32)
                nc.sync.dma_start(dht[:], dht_d.ap())
            # s2t rides the SWDGE queue so it never queues behind w1x chunks
            s2t = const.tile([128, 8 * B], bf16)
            nc.gpsimd.dma_start(s2t[:], s2t_d.ap())

            at_sb = acc.tile([128, 8 * JSH], bf16)
            ph = psH.tile([JSH, B], f32)

            dma_engines = [nc.sync, nc.scalar]

            for bk in range(NCHUNK):
                w1c = w1pool.tile([128, 2, CHCOL], fp8, tag="w1c")
                dma_engines[bk % 2].dma_start(
                    w1c[:], w1x_d.ap()[:, :, CHCOL * bk : CHCOL * (bk + 1)]
                )
                pa = psA.tile([32, 512], f32, name="pa", tag="pa")
                for p8 in range(8):
                    nc.tensor.matmul(
                        pa[:, 0:500],
                        ones8[:],
                        w1c[:, :, 500 * p8 : 500 * (p8 + 1)],
                        start=(p8 == 0),
                        stop=(p8 == 7),
                        perf_mode=DR,
                    )
                # psum [32, 500] -> SBUF bf16, then reshape-DMA to the
                # [128, 125] stage-2 lhsT tile (partition p = 4m + c4)
                s4 = s4pool.tile([32, 500], bf16, tag="s4")
                if bk % 2 == 0:
                    nc.vector.tensor_copy(s4[:], pa[:, 0:500])
                else:
                    nc.scalar.activation(
                        s4[:], pa[:, 0:500], mybir.ActivationFunctionType.Copy
                    )
                nc.gpsimd.dma_start(at_sb[:, JSH * bk : JSH * (bk + 1)], s4[:])
                nc.tensor.matmul(
                    ph[:, :],
                    at_sb[:, JSH * bk : JSH * (bk + 1)],
                    s2t[:, B * bk : B * (bk + 1)],
                    start=(bk == 0),
                    stop=(bk == 7),
                )

            if use_dh:
                nc.vector.tensor_add(ph[:, :], ph[:, :], dht[:, :])
            h_sb = acc.tile([JSH, B], f32)
            nc.scalar.activation(
                h_sb[:, :],
                ph[:, :],
                mybir.ActivationFunctionType.Tanh,
                bias=bias[:, 0:1],
                scale=float(1.0 / SCALE),
            )

            # ---- stage 3: partial[k, b] = sum_j l2t[j, k] h[j, b] ----
            po = psO.tile([L, B], f32)
            nc.tensor.matmul(po[:, :], l2t[:, :], h_sb[:, :], start=True, stop=True)
            o_sb = acc.tile([L, B], f32)
            nc.vector.tensor_copy(o_sb[:, :], po[:, :])
            nc.sync.dma_start(outp_d.ap(), o_sb[:, :])

    nc.compile()
    _CACHE[key] = nc
    return nc


def _quantize_feedback(w1p):
    """fp8e4 (TRN E4M3) quantization with error feedback along the last axis.

    Carrying each element's rounding error into the next keeps the sum over
    the last axis accurate to ~1 ulp of a single element.
    """
    import ml_dtypes

    fp8 = ml_dtypes.float8_e4m3
    q = np.empty(w1p.shape, dtype=fp8)
    err = np.zeros(w1p.shape[:-1], np.float32)
    for l in range(w1p.shape[-1]):
        v = w1p[..., l] + err
        ql = np.clip(v, -240, 240).astype(fp8)
        q[..., l] = ql
        err = v - ql.astype(np.float32)
    return q


def profile_last(trace_cores=None):
    """Re-run the last-built program with NTFF tracing (dev/test helper)."""
    if "last_run" not in _CACHE:
        return None
    from concourse.bass_utils import run_bass_kernel_spmd

    nc, in_maps = _CACHE["last_run"]
    return run_bass_kernel_spmd(
        nc,
        in_maps,
        list(range(NCORES)),
        trace=True,
        trace_cores=trace_cores,
    )


def _host_prep(
    params,
    wfft0_re,
    wfft0_im,
    wfft1_re,
    wfft1_im,
    wfft2_re,
    wfft2_im,
    lin1_w,
    lin1_b,
    lin2_w,
    lin2_b,
):
    """All host-side prep: collapse, quantize, per-core shards."""
    import ml_dtypes

    fp8 = ml_dtypes.float8_e4m3
    bf16 = ml_dtypes.bfloat16

    a = params[:, :, 0].astype(np.float64)
    w0 = _wvec(wfft0_re, wfft0_im)[0]
    w1v = _wvec(wfft1_re, wfft1_im)[0]
    w2 = _wvec(wfft2_re, wfft2_im)
    s1 = _retanh(a, w0)
    s2 = _retanh(s1, w1v)                          # (B, MODES) f64
    x1 = _retanh(np.float64(1.0), w2)              # (64,) f64

    # s2 as the device will see it (bf16 in the stage-2 rhs)
    s2q = s2.astype(np.float32).astype(bf16).astype(np.float32)

    # exact correction for entries where tanh did not saturate to +-1
    bad_b, bad_f = np.nonzero(np.abs(s2q) != np.float32(1.0))
    use_dh = bad_b.size > 0
    dh = None
    if use_dh:
        dh = np.zeros((B, MODES), np.float64)
        x1_64 = x1.astype(np.float64)
        for bb, ff in zip(bad_b.tolist(), bad_f.tolist()):
            sdev = np.float64(s2q[bb, ff])
            delta = _retanh(s2[bb, ff], w2) - sdev * x1_64   # (64,)
            dh[bb, :] += lin1_w[:, 64 * ff : 64 * (ff + 1)].astype(np.float64) @ delta
        dh = (dh * SCALE).astype(np.float32)       # pre-scaled like the psum

    # ---- fold X1 into lin1_w, quantize fp8 with error feedback ----
    w1p = (
        lin1_w.reshape(MODES, MODES, L).astype(np.float32)
        * x1[None, None, :].astype(np.float32)
    ) * np.float32(SCALE)                          # (o, f, l)
    w1q = _quantize_feedback(w1p)                  # (o, f, l) fp8
    del w1p

    # s2t layout: [128 p, 8 B, 512 b]; tile B row p holds f = 128B + 32*(p%4) + p//4
    # (that is where the [32,500]->[128,125] reshape DMA puts A[f, :])
    p_idx = np.arange(128)
    f_of_p = 32 * (p_idx % 4) + p_idx // 4          # within-tile f offset
    s2f = np.zeros((FPAD, B), np.float32)
    s2f[:MODES] = s2q.T
    s2t = np.zeros((128, 8, B), bf16)
    for t in range(8):
        s2t[:, t, :] = s2f[128 * t + f_of_p].astype(bf16)
    s2t = np.ascontiguousarray(s2t.reshape(128, 8 * B))

    # ones block-diagonal stationary operand: ones8[p, i, m] = (m == 16i + p//8)
    ones8 = np.zeros((128, 2, 32), np.float32)
    for p in range(128):
        for i in range(2):
            ones8[p, i, 16 * i + p // 8] = 1.0
    ones8 = ones8.astype(fp8)

    in_maps = []
    for c in range(NCORES):
        j0, j1 = JSH * c, JSH * (c + 1)
        # w1x[p, i, (B, P, c4, j)]:
        #   f = 128B + 32c4 + 16i + p//8,  l = 8P + p%8
        qp = np.zeros((JSH, FPAD, L), fp8)
        qp[:, :MODES, :] = w1q[j0:j1]
        w1x = np.ascontiguousarray(
            qp.reshape(JSH, 8, 4, 2, 16, 8, 8)         # j B c4 i mh P dl
            .transpose(4, 6, 3, 1, 5, 2, 0)            # mh dl i B P c4 j
            .reshape(128, 2, NCHUNK * CHCOL)
        )
        m = {
            "w1x": w1x,
            "s2t": s2t,
            "ones8": ones8,
            "bias": np.ascontiguousarray(lin1_b[j0:j1].reshape(JSH, 1)),
            "l2t": np.ascontiguousarray(lin2_w[:, j0:j1].T),
        }
        if use_dh:
            m["dht"] = np.ascontiguousarray(dh[:, j0:j1].T)
        in_maps.append(m)
    return use_dh, in_maps


def kernel(
    params,
    wfft0_re,
    wfft0_im,
    wfft1_re,
    wfft1_im,
    wfft2_re,
    wfft2_im,
    lin1_w,
    lin1_b,
    lin2_w,
    lin2_b,
):
    from concourse.bass_utils import run_bass_kernel_spmd

    use_dh, in_maps = _host_prep(
        params, wfft0_re, wfft0_im, wfft1_re, wfft1_im, wfft2_re, wfft2_im,
        lin1_w, lin1_b, lin2_w, lin2_b,
    )
    nc = _build_program(use_dh)
    _CACHE["last_run"] = (nc, in_maps)
    res = run_bass_kernel_spmd(nc, in_maps, list(range(NCORES)))

    acc = np.zeros((L, B), np.float64)
    for c in range(NCORES):
        acc += res.results[c]["outp"].astype(np.float64)
    out = 1.0 / (1.0 + np.exp(-(acc.T + lin2_b.astype(np.float64))))
    return out.astype(np.float32)


# revision 8
# speedup vs baseline: 1.7400x; 1.0280x over previous
"""Trainium2 Bass kernel for nn_Metamorph_parameterReinforcer.

Math background (exact identities, verified against the reference):
  The reference's einsum("bfp,mn->bfm", fx, wfft) sums over BOTH p and n,
  so each "STFT block" collapses:
    sum_p fft(x, norm=forward)[..., p] == x[..., 0]
    block(x)[b, f, k] = Re tanh(x[b, f, 0] * W[k]),
       W[k] = sum_m (sum_n wfft[m, n]) * exp(2j*pi*k*m/64)
  Chaining three blocks, only element 0 of the last axis propagates:
    a  = params[:, :, 0]
    s1 = Retanh(a  * W0[0]);  s2 = Retanh(s1 * W1[0])
    x3[b, f, l] = Retanh(s2[b, f] * W2[l])         # (512, 1000, 64)
    h  = tanh(x3.reshape(512, 64000) @ lin1_w.T + lin1_b)
    out = sigmoid(h @ lin2_w.T + lin2_b)
  Because |W0[0]|, |W1[0]| ~ 32000, tanh saturates and s2 is +-1 in f32 for
  all but (rare) tiny-|a| entries; where s2 = +-1, x3[b, f, :] =
  s2[b, f] * X1[:] with X1 = Retanh(W2) -- exactly rank-1.  Rare
  non-saturated entries get an exact correction (dht) added before the lin1
  tanh.

Device kernel (8 cores, lin1_w sharded over its output dim j, 125 rows/core).
v2 design (fp8 + DoubleRow + constant stationary operand):
  * X1 is folded into lin1_w on the host: w1'[o, f, l] = lin1_w[o, 64f+l] *
    X1[l].  Stage 1 then reduces w1' over l with a CONSTANT ones
    block-diagonal stationary operand, so every matmul shares one lhsT and
    columns from different f-groups ride in the same matmul.
  * w1' is quantized to fp8e4 (TRN E4M3, max 240) with ERROR-FEEDBACK along
    l: the rounding error of element l is carried into element l+1, so the
    per-(o,f) SUM over l is accurate to ~1 ulp instead of sqrt(64) ulps.
    Simulated end-to-end rel err 2.4e-3 (vs 1.7e-2 naive fp8, 1.5e-3 bf16).
  * DoubleRow perf mode: virtual K=256 (2 fp8 per PE cell), halving both
    DMA bytes (8.4 MB/core) and PE streaming time vs bf16.  DoubleRow
    outputs must sit at psum partition 0 (walrus ISA check), so:
  * f padded 1000->1024, split into 8 banks of 128.  Bank B accumulates
    psum [32, 500] over 8 passes: psum[m, 125*c4 + j] = A[f, j] with
    f = 128B + 32*c4 + m.  One chunk = one bank's full fp8 stream (1 MB).
  * Per bank: DVE/ScalarE copy [32, 500] psum->SBUF (bf16), then ONE
    gpsimd SBUF->SBUF DMA reshapes [32, 500] -> [128, 125] (partition p =
    4m + c4; the host permutes s2t rows to match), giving the [128 f, 125 j]
    lhsT tile for stage 2.  All per-bank work overlaps later banks' stream.
  * stage 2: h[j, b] = tanh(2^-11 * sum_f A[f, j] s2[f, b] + bias) -- 8
    accumulating matmuls (K = 128 f's each) + ScalarE tanh (the activation
    scale folds the fp8 2^11 quantization scale away).
  * stage 3: partial[k, b] = sum_j l2t[j, k] h[j, b]  (one matmul)
Host combines the 8 partials: out = sigmoid(sum_c partial_c + lin2_b).
"""

import numpy as np

B, MODES, L = 512, 1000, 64
NCORES = 8
JSH = MODES // NCORES          # 125 lin1 output rows per core
FPAD = 1024                    # padded mode count (8 stage-2 K-tiles of 128)
NCHUNK = 8                     # w1x DMA chunks = psum banks (1 MB each)
CHCOL = 4000                   # columns per chunk: (P, c4, j) = 8*4*125
SCALE = 2.0 ** 11              # fp8 quantization scale for w1'
SAT = 50.0                     # |2*s*Re(W)| beyond this: Retanh == sign


def _retanh(s, w):
    """Re tanh(s * w) for real array s and complex (array or scalar) w."""
    s = np.asarray(s, np.float64)
    x = 2.0 * np.multiply.outer(s, np.real(w))
    y = 2.0 * np.multiply.outer(s, np.imag(w))
    xc = np.clip(x, -SAT, SAT)
    with np.errstate(over="ignore", invalid="ignore"):
        r = np.sinh(xc) / (np.cosh(xc) + np.cos(y))
    return np.where(np.abs(x) >= SAT, np.sign(x), r)


def _wvec(wre, wim):
    """W[k] = sum_m (sum_n w[m, n]) * exp(2j pi k m / L)."""
    wsum = wre.astype(np.float64).sum(axis=1) + 1j * wim.astype(np.float64).sum(axis=1)
    tw = np.exp(2j * np.pi * np.outer(np.arange(L), np.arange(L)) / L)
    return tw @ wsum


_CACHE = {}


def _build_program(use_dh):
    """Build (and cache) the Bass program. Same program for all 8 cores."""
    key = ("prog", use_dh, "fp8dr_v3")
    if key in _CACHE:
        return _CACHE[key]

    import concourse.bacc as bacc
    import concourse.mybir as mybir
    import concourse.tile as tile

    f32 = mybir.dt.float32
    f32r = mybir.dt.float32r
    bf16 = mybir.dt.bfloat16
    fp8 = mybir.dt.float8e4
    DR = mybir.MatmulPerfMode.DoubleRow
    nc = bacc.Bacc("TRN2", target_bir_lowering=False, debug=False)

    # [p, bank, i, col]: each (p, bank) is one contiguous 8 KB run in DRAM
    w1x_d = nc.dram_tensor("w1x", [128, NCHUNK, 2, CHCOL], fp8, kind="ExternalInput")
    s2t_d = nc.dram_tensor("s2t", [128, 8 * B], bf16, kind="ExternalInput")
    ones_d = nc.dram_tensor("ones8", [128, 2, 32], fp8, kind="ExternalInput")
    bias_d = nc.dram_tensor("bias", [JSH, 1], f32, kind="ExternalInput")
    l2t_d = nc.dram_tensor("l2t", [JSH, L], f32r, kind="ExternalInput")
    if use_dh:
        dht_d = nc.dram_tensor("dht", [JSH, B], f32, kind="ExternalInput")
    outp_d = nc.dram_tensor("outp", [L, B], f32, kind="ExternalOutput")

    # sub-DMA split per bank (in 1000-column = 2-pass units): the first and
    # last banks use 4x256KB so the PE ramps early and the tail chain starts
    # before the final bytes land; middle banks use full 1 MB transfers.
    SPLITS = {0: [1, 1, 1, 1], 1: [2, 2], 6: [2, 2], 7: [2, 1, 1]}

    with tile.TileContext(nc) as tc:
        with (
            tc.tile_pool(name="const", bufs=1) as const,
            tc.tile_pool(name="w1pool", bufs=3) as w1pool,
            tc.tile_pool(name="s4pool", bufs=2) as s4pool,
            tc.tile_pool(name="acc", bufs=1) as acc,
            tc.tile_pool(name="psA", bufs=3, space="PSUM") as psA,
            tc.tile_pool(name="psH", bufs=1, space="PSUM") as psH,
            tc.tile_pool(name="psO", bufs=1, space="PSUM") as psO,
        ):
            # ones8 first on the sync ring (needed by the first LDWEIGHTS);
            # all other consts are issued AFTER the w1x chunks (they are only
            # needed by stage 2/3).  s2t rides the SWDGE queue.
            ones8 = const.tile([128, 2, 32], fp8)
            nc.sync.dma_start(ones8[:], ones_d.ap())
            s2t = const.tile([128, 8 * B], bf16)
            nc.gpsimd.dma_start(s2t[:], s2t_d.ap())

            at_sb = acc.tile([128, 8 * JSH], bf16)
            ph = psH.tile([JSH, B], f32)

            dma_engines = [nc.sync, nc.scalar]

            for bk in range(NCHUNK):
                eng = dma_engines[bk % 2]
                w1c = w1pool.tile([128, 2, CHCOL], fp8, tag="w1c")
                c0 = 0
                for units in SPLITS.get(bk, [4]):
                    cw = 1000 * units
                    eng.dma_start(
                        w1c[:, :, c0 : c0 + cw],
                        w1x_d.ap()[:, bk, :, c0 : c0 + cw],
                    )
                    c0 += cw
                pa = psA.tile([32, 512], f32, name="pa", tag="pa")
                for p8 in range(8):
                    nc.tensor.matmul(
                        pa[:, 0:500],
                        ones8[:],
                        w1c[:, :, 500 * p8 : 500 * (p8 + 1)],
                        start=(p8 == 0),
                        stop=(p8 == 7),
                        perf_mode=DR,
                    )
                # psum [32, 500] -> SBUF bf16, then reshape-DMA to the
                # [128, 125] stage-2 lhsT tile (partition p = 4m + c4)
                s4 = s4pool.tile([32, 500], bf16, tag="s4")
                if bk % 2 == 0:
                    nc.vector.tensor_copy(s4[:], pa[:, 0:500])
                else:
                    nc.scalar.activation(
                        s4[:], pa[:, 0:500], mybir.ActivationFunctionType.Copy
                    )
                nc.gpsimd.dma_start(at_sb[:, JSH * bk : JSH * (bk + 1)], s4[:])
                nc.tensor.matmul(
                    ph[:, :],
                    at_sb[:, JSH * bk : JSH * (bk + 1)],
                    s2t[:, B * bk : B * (bk + 1)],
                    start=(bk == 0),
                    stop=(bk == 7),
                )

            # late consts (needed only for the activation / stage 3)
            bias = const.tile([JSH, 1], f32)
            nc.sync.dma_start(bias[:], bias_d.ap())
            l2t = const.tile([JSH, L], f32r)
            nc.sync.dma_start(l2t[:], l2t_d.ap())
            if use_dh:
                dht = const.tile([JSH, B], f32)
                nc.sync.dma_start(dht[:], dht_d.ap())
                nc.vector.tensor_add(ph[:, :], ph[:, :], dht[:, :])
            h_sb = acc.tile([JSH, B], f32r)
            nc.scalar.activation(
                h_sb[:, :],
                ph[:, :],
                mybir.ActivationFunctionType.Tanh,
                bias=bias[:, 0:1],
                scale=float(1.0 / SCALE),
            )

            # ---- stage 3: partial[k, b] = sum_j l2t[j, k] h[j, b] ----
            po = psO.tile([L, B], f32)
            nc.tensor.matmul(po[:, :], l2t[:, :], h_sb[:, :], start=True, stop=True)
            o_sb = acc.tile([L, B], f32)
            nc.vector.tensor_copy(o_sb[:, :], po[:, :])
            nc.sync.dma_start(outp_d.ap(), o_sb[:, :])

    nc.compile()
    _CACHE[key] = nc
    return nc


def _quantize_feedback(w1p):
    """fp8e4 (TRN E4M3) quantization with error feedback along the last axis.

    Carrying each element's rounding error into the next keeps the sum over
    the last axis accurate to ~1 ulp of a single element.
    """
    import ml_dtypes

    fp8 = ml_dtypes.float8_e4m3
    q = np.empty(w1p.shape, dtype=fp8)
    err = np.zeros(w1p.shape[:-1], np.float32)
    for l in range(w1p.shape[-1]):
        v = w1p[..., l] + err
        ql = np.clip(v, -240, 240).astype(fp8)
        q[..., l] = ql
        err = v - ql.astype(np.float32)
    return q


def profile_last(trace_cores=None):
    """Re-run the last-built program with NTFF tracing (dev/test helper)."""
    if "last_run" not in _CACHE:
        return None
    from concourse.bass_utils import run_bass_kernel_spmd

    nc, in_maps = _CACHE["last_run"]
    return run_bass_kernel_spmd(
        nc,
        in_maps,
        list(range(NCORES)),
        trace=True,
        trace_cores=trace_cores,
    )


def _host_prep(
    params,
    wfft0_re,
    wfft0_im,
    wfft1_re,
    wfft1_im,
    wfft2_re,
    wfft2_im,
    lin1_w,
    lin1_b,
    lin2_w,
    lin2_b,
):
    """All host-side prep: collapse, quantize, per-core shards."""
    import ml_dtypes

    fp8 = ml_dtypes.float8_e4m3
    bf16 = ml_dtypes.bfloat16

    a = params[:, :, 0].astype(np.float64)
    w0 = _wvec(wfft0_re, wfft0_im)[0]
    w1v = _wvec(wfft1_re, wfft1_im)[0]
    w2 = _wvec(wfft2_re, wfft2_im)
    s1 = _retanh(a, w0)
    s2 = _retanh(s1, w1v)                          # (B, MODES) f64
    x1 = _retanh(np.float64(1.0), w2)              # (64,) f64

    # s2 as the device will see it (bf16 in the stage-2 rhs)
    s2q = s2.astype(np.float32).astype(bf16).astype(np.float32)

    # exact correction for entries where tanh did not saturate to +-1
    bad_b, bad_f = np.nonzero(np.abs(s2q) != np.float32(1.0))
    use_dh = bad_b.size > 0
    dh = None
    if use_dh:
        dh = np.zeros((B, MODES), np.float64)
        x1_64 = x1.astype(np.float64)
        for bb, ff in zip(bad_b.tolist(), bad_f.tolist()):
            sdev = np.float64(s2q[bb, ff])
            delta = _retanh(s2[bb, ff], w2) - sdev * x1_64   # (64,)
            dh[bb, :] += lin1_w[:, 64 * ff : 64 * (ff + 1)].astype(np.float64) @ delta
        dh = (dh * SCALE).astype(np.float32)       # pre-scaled like the psum

    # ---- fold X1 into lin1_w, quantize fp8 with error feedback ----
    w1p = (
        lin1_w.reshape(MODES, MODES, L).astype(np.float32)
        * x1[None, None, :].astype(np.float32)
    ) * np.float32(SCALE)                          # (o, f, l)
    w1q = _quantize_feedback(w1p)                  # (o, f, l) fp8
    del w1p

    # s2t layout: [128 p, 8 B, 512 b]; tile B row p holds f = 128B + 32*(p%4) + p//4
    # (that is where the [32,500]->[128,125] reshape DMA puts A[f, :])
    p_idx = np.arange(128)
    f_of_p = 32 * (p_idx % 4) + p_idx // 4          # within-tile f offset
    s2f = np.zeros((FPAD, B), np.float32)
    s2f[:MODES] = s2q.T
    s2t = np.zeros((128, 8, B), bf16)
    for t in range(8):
        s2t[:, t, :] = s2f[128 * t + f_of_p].astype(bf16)
    s2t = np.ascontiguousarray(s2t.reshape(128, 8 * B))

    # ones block-diagonal stationary operand: ones8[p, i, m] = (m == 16i + p//8)
    ones8 = np.zeros((128, 2, 32), np.float32)
    for p in range(128):
        for i in range(2):
            ones8[p, i, 16 * i + p // 8] = 1.0
    ones8 = ones8.astype(fp8)

    in_maps = []
    for c in range(NCORES):
        j0, j1 = JSH * c, JSH * (c + 1)
        # w1x[p, B, i, (P, c4, j)]:
        #   f = 128B + 32c4 + 16i + p//8,  l = 8P + p%8
        qp = np.zeros((JSH, FPAD, L), fp8)
        qp[:, :MODES, :] = w1q[j0:j1]
        w1x = np.ascontiguousarray(
            qp.reshape(JSH, 8, 4, 2, 16, 8, 8)         # j B c4 i mh P dl
            .transpose(4, 6, 1, 3, 5, 2, 0)            # mh dl B i P c4 j
            .reshape(128, NCHUNK, 2, CHCOL)
        )
        m = {
            "w1x": w1x,
            "s2t": s2t,
            "ones8": ones8,
            "bias": np.ascontiguousarray(lin1_b[j0:j1].reshape(JSH, 1)),
            "l2t": np.ascontiguousarray(lin2_w[:, j0:j1].T),
        }
        if use_dh:
            m["dht"] = np.ascontiguousarray(dh[:, j0:j1].T)
        in_maps.append(m)
    return use_dh, in_maps


def kernel(
    params,
    wfft0_re,
    wfft0_im,
    wfft1_re,
    wfft1_im,
    wfft2_re,
    wfft2_im,
    lin1_w,
    lin1_b,
    lin2_w,
    lin2_b,
):
    from concourse.bass_utils import run_bass_kernel_spmd

    use_dh, in_maps = _host_prep(
        params, wfft0_re, wfft0_im, wfft1_re, wfft1_im, wfft2_re, wfft2_im,
        lin1_w, lin1_b, lin2_w, lin2_b,
    )
    nc = _build_program(use_dh)
    _CACHE["last_run"] = (nc, in_maps)
    res = run_bass_kernel_spmd(nc, in_maps, list(range(NCORES)))

    acc = np.zeros((L, B), np.float64)
    for c in range(NCORES):
        acc += res.results[c]["outp"].astype(np.float64)
    out = 1.0 / (1.0 + np.exp(-(acc.T + lin2_b.astype(np.float64))))
    return out.astype(np.float32)


# revision 14
# speedup vs baseline: 1.8421x; 1.0587x over previous
"""Trainium2 Bass kernel for nn_Metamorph_parameterReinforcer.

Math background (exact identities, verified against the reference):
  The reference's einsum("bfp,mn->bfm", fx, wfft) sums over BOTH p and n,
  so each "STFT block" collapses:
    sum_p fft(x, norm=forward)[..., p] == x[..., 0]
    block(x)[b, f, k] = Re tanh(x[b, f, 0] * W[k]),
       W[k] = sum_m (sum_n wfft[m, n]) * exp(2j*pi*k*m/64)
  Chaining three blocks, only element 0 of the last axis propagates:
    a  = params[:, :, 0]
    s1 = Retanh(a  * W0[0]);  s2 = Retanh(s1 * W1[0])
    x3[b, f, l] = Retanh(s2[b, f] * W2[l])         # (512, 1000, 64)
    h  = tanh(x3.reshape(512, 64000) @ lin1_w.T + lin1_b)
    out = sigmoid(h @ lin2_w.T + lin2_b)
  Because |W0[0]|, |W1[0]| ~ 32000, tanh saturates and s2 is +-1 in f32 for
  all but (rare) tiny-|a| entries; where s2 = +-1, x3[b, f, :] =
  s2[b, f] * X1[:] with X1 = Retanh(W2) -- exactly rank-1.  Rare
  non-saturated entries get an exact correction (dht) added before the lin1
  tanh.

Device kernel (8 cores, lin1_w sharded over its output dim j, 125 rows/core).
v2 design (fp8 + DoubleRow + constant stationary operand):
  * X1 is folded into lin1_w on the host: w1'[o, f, l] = lin1_w[o, 64f+l] *
    X1[l].  Stage 1 then reduces w1' over l with a CONSTANT ones
    block-diagonal stationary operand, so every matmul shares one lhsT and
    columns from different f-groups ride in the same matmul.
  * w1' is quantized to fp8e4 (TRN E4M3, max 240) with ERROR-FEEDBACK along
    l: the rounding error of element l is carried into element l+1, so the
    per-(o,f) SUM over l is accurate to ~1 ulp instead of sqrt(64) ulps.
    Simulated end-to-end rel err 2.4e-3 (vs 1.7e-2 naive fp8, 1.5e-3 bf16).
  * DoubleRow perf mode: virtual K=256 (2 fp8 per PE cell), halving both
    DMA bytes (8.4 MB/core) and PE streaming time vs bf16.  DoubleRow
    outputs must sit at psum partition 0 (walrus ISA check), so:
  * f padded 1000->1024, split into 8 banks of 128.  Bank B accumulates
    psum [32, 500] over 8 passes: psum[m, 125*c4 + j] = A[f, j] with
    f = 128B + 32*c4 + m.  One chunk = one bank's full fp8 stream (1 MB).
  * Per bank: DVE/ScalarE copy [32, 500] psum->SBUF (bf16), then ONE
    gpsimd SBUF->SBUF DMA reshapes [32, 500] -> [128, 125] (partition p =
    4m + c4; the host permutes s2t rows to match), giving the [128 f, 125 j]
    lhsT tile for stage 2.  All per-bank work overlaps later banks' stream.
  * stage 2: h[j, b] = tanh(2^-11 * sum_f A[f, j] s2[f, b] + bias) -- 8
    accumulating matmuls (K = 128 f's each) + ScalarE tanh (the activation
    scale folds the fp8 2^11 quantization scale away).
  * stage 3: partial[k, b] = sum_j l2t[j, k] h[j, b]  (one matmul)
Host combines the 8 partials: out = sigmoid(sum_c partial_c + lin2_b).
"""

import numpy as np

B, MODES, L = 512, 1000, 64
NCORES = 8
JSH = MODES // NCORES          # 125 lin1 output rows per core
FPAD = 1024                    # padded mode count (8 stage-2 K-tiles of 128)
NCHUNK = 8                     # w1x DMA chunks = psum banks (1 MB each)
CHCOL = 4000                   # columns per chunk: (P, c4, j) = 8*4*125
SCALE = 2.0 ** 11              # fp8 quantization scale for w1'
SAT = 50.0                     # |2*s*Re(W)| beyond this: Retanh == sign


def _retanh(s, w):
    """Re tanh(s * w) for real array s and complex (array or scalar) w."""
    s = np.asarray(s, np.float64)
    x = 2.0 * np.multiply.outer(s, np.real(w))
    y = 2.0 * np.multiply.outer(s, np.imag(w))
    xc = np.clip(x, -SAT, SAT)
    with np.errstate(over="ignore", invalid="ignore"):
        r = np.sinh(xc) / (np.cosh(xc) + np.cos(y))
    return np.where(np.abs(x) >= SAT, np.sign(x), r)


def _wvec(wre, wim):
    """W[k] = sum_m (sum_n w[m, n]) * exp(2j pi k m / L)."""
    wsum = wre.astype(np.float64).sum(axis=1) + 1j * wim.astype(np.float64).sum(axis=1)
    tw = np.exp(2j * np.pi * np.outer(np.arange(L), np.arange(L)) / L)
    return tw @ wsum


_CACHE = {}


def _build_program(use_dh):
    """Build (and cache) the Bass program. Same program for all 8 cores."""
    key = ("prog", use_dh, "fp8dr_v4")
    if key in _CACHE:
        return _CACHE[key]

    import concourse.bacc as bacc
    import concourse.mybir as mybir
    import concourse.tile as tile

    f32 = mybir.dt.float32
    f32r = mybir.dt.float32r
    bf16 = mybir.dt.bfloat16
    fp8 = mybir.dt.float8e4
    DR = mybir.MatmulPerfMode.DoubleRow
    nc = bacc.Bacc("TRN2", target_bir_lowering=False, debug=False)

    # [p, bank, i, col]: each (p, bank) is one contiguous 8 KB run in DRAM
    w1x_d = nc.dram_tensor("w1x", [128, NCHUNK, 2, CHCOL], fp8, kind="ExternalInput")
    s2t_d = nc.dram_tensor("s2t", [128, 8 * B], fp8, kind="ExternalInput")
    ones_d = nc.dram_tensor("ones8", [128, 2, 32], fp8, kind="ExternalInput")
    bias_d = nc.dram_tensor("bias", [JSH, 1], f32, kind="ExternalInput")
    l2t_d = nc.dram_tensor("l2t", [JSH, L], f32r, kind="ExternalInput")
    if use_dh:
        dht_d = nc.dram_tensor("dht", [JSH, B], f32, kind="ExternalInput")
    outp_d = nc.dram_tensor("outp", [L, B], f32, kind="ExternalOutput")

    # sub-DMA split per bank (in 1000-column = 2-pass units).  Descriptor
    # runs are (split bytes)/256 per (p, i) line; HWDGE descriptor-gen rate
    # (~1 desc / 20 ns / ring) makes runs under ~4 KB ring-starved, so
    # 512 KB (2 units) is the smallest useful split.  First bank split so
    # the PE starts earlier; last bank split so the tail chain starts
    # before the final bytes land.
    SPLITS = {0: [2, 2], 7: [2, 2]}

    with tile.TileContext(nc) as tc:
        with (
            tc.tile_pool(name="const", bufs=1) as const,
            tc.tile_pool(name="w1pool", bufs=3) as w1pool,
            tc.tile_pool(name="s4pool", bufs=2) as s4pool,
            tc.tile_pool(name="acc", bufs=1) as acc,
            tc.tile_pool(name="psA", bufs=3, space="PSUM") as psA,
            tc.tile_pool(name="psH", bufs=1, space="PSUM") as psH,
            tc.tile_pool(name="psO", bufs=1, space="PSUM") as psO,
        ):
            # ones8 first on the sync ring (needed by the first LDWEIGHTS);
            # all other consts are issued AFTER the w1x chunks (they are only
            # needed by stage 2/3).  s2t rides the SWDGE queue.
            ones8 = const.tile([128, 2, 32], fp8)
            nc.sync.dma_start(ones8[:], ones_d.ap())
            s2t = const.tile([128, 8 * B], fp8)
            nc.gpsimd.dma_start(s2t[:], s2t_d.ap())

            at_sb = acc.tile([128, 8 * JSH], bf16)
            ph = psH.tile([JSH, B], f32)

            dma_engines = [nc.sync, nc.scalar]

            for bk in range(NCHUNK):
                eng = dma_engines[bk % 2]
                w1c = w1pool.tile([128, 2, CHCOL], fp8, tag="w1c")
                c0 = 0
                for units in SPLITS.get(bk, [4]):
                    cw = 1000 * units
                    eng.dma_start(
                        w1c[:, :, c0 : c0 + cw],
                        w1x_d.ap()[:, bk, :, c0 : c0 + cw],
                    )
                    c0 += cw
                pa = psA.tile([32, 512], f32, name="pa", tag="pa")
                for p8 in range(8):
                    nc.tensor.matmul(
                        pa[:, 0:500],
                        ones8[:],
                        w1c[:, :, 500 * p8 : 500 * (p8 + 1)],
                        start=(p8 == 0),
                        stop=(p8 == 7),
                        perf_mode=DR,
                    )
                # psum [32, 500] -> SBUF bf16, then reshape-DMA to the
                # [128, 125] stage-2 lhsT tile (partition p = 4m + c4)
                s4 = s4pool.tile([32, 500], bf16, tag="s4")
                if bk % 2 == 0:
                    nc.vector.tensor_copy(s4[:], pa[:, 0:500])
                else:
                    nc.scalar.activation(
                        s4[:], pa[:, 0:500], mybir.ActivationFunctionType.Copy
                    )
                nc.gpsimd.dma_start(at_sb[:, JSH * bk : JSH * (bk + 1)], s4[:])
                nc.tensor.matmul(
                    ph[:, :],
                    at_sb[:, JSH * bk : JSH * (bk + 1)],
                    s2t[:, B * bk : B * (bk + 1)],
                    start=(bk == 0),
                    stop=(bk == 7),
                )

            # late consts (needed only for the activation / stage 3)
            bias = const.tile([JSH, 1], f32)
            nc.sync.dma_start(bias[:], bias_d.ap())
            l2t = const.tile([JSH, L], f32r)
            nc.sync.dma_start(l2t[:], l2t_d.ap())
            if use_dh:
                dht = const.tile([JSH, B], f32)
                nc.sync.dma_start(dht[:], dht_d.ap())
                nc.vector.tensor_add(ph[:, :], ph[:, :], dht[:, :])
            h_sb = acc.tile([JSH, B], f32r)
            nc.scalar.activation(
                h_sb[:, :],
                ph[:, :],
                mybir.ActivationFunctionType.Tanh,
                bias=bias[:, 0:1],
                scale=float(1.0 / SCALE),
            )

            # ---- stage 3: partial[k, b] = sum_j l2t[j, k] h[j, b] ----
            po = psO.tile([L, B], f32)
            nc.tensor.matmul(po[:, :], l2t[:, :], h_sb[:, :], start=True, stop=True)
            o_sb = acc.tile([L, B], f32)
            nc.vector.tensor_copy(o_sb[:, :], po[:, :])
            nc.sync.dma_start(outp_d.ap(), o_sb[:, :])

    nc.compile()
    _CACHE[key] = nc
    return nc


def _quantize_feedback(w1p):
    """fp8e4 (TRN E4M3) quantization with error feedback along the last axis.

    Carrying each element's rounding error into the next keeps the sum over
    the last axis accurate to ~1 ulp of a single element.
    """
    import ml_dtypes

    fp8 = ml_dtypes.float8_e4m3
    q = np.empty(w1p.shape, dtype=fp8)
    err = np.zeros(w1p.shape[:-1], np.float32)
    for l in range(w1p.shape[-1]):
        v = w1p[..., l] + err
        ql = np.clip(v, -240, 240).astype(fp8)
        q[..., l] = ql
        err = v - ql.astype(np.float32)
    return q


def profile_last(trace_cores=None):
    """Re-run the last-built program with NTFF tracing (dev/test helper)."""
    if "last_run" not in _CACHE:
        return None
    from concourse.bass_utils import run_bass_kernel_spmd

    nc, in_maps = _CACHE["last_run"]
    return run_bass_kernel_spmd(
        nc,
        in_maps,
        list(range(NCORES)),
        trace=True,
        trace_cores=trace_cores,
    )


def _host_prep(
    params,
    wfft0_re,
    wfft0_im,
    wfft1_re,
    wfft1_im,
    wfft2_re,
    wfft2_im,
    lin1_w,
    lin1_b,
    lin2_w,
    lin2_b,
):
    """All host-side prep: collapse, quantize, per-core shards."""
    import ml_dtypes

    fp8 = ml_dtypes.float8_e4m3
    bf16 = ml_dtypes.bfloat16

    a = params[:, :, 0].astype(np.float64)
    w0 = _wvec(wfft0_re, wfft0_im)[0]
    w1v = _wvec(wfft1_re, wfft1_im)[0]
    w2 = _wvec(wfft2_re, wfft2_im)
    s1 = _retanh(a, w0)
    s2 = _retanh(s1, w1v)                          # (B, MODES) f64
    x1 = _retanh(np.float64(1.0), w2)              # (64,) f64

    # s2 as the device will see it (fp8 in the stage-2 rhs; +-1 is exact)
    s2q = np.clip(s2, -240, 240).astype(fp8).astype(np.float32)

    # exact correction for entries where tanh did not saturate to +-1
    bad_b, bad_f = np.nonzero(np.abs(s2q) != np.float32(1.0))
    use_dh = bad_b.size > 0
    dh = None
    if use_dh:
        dh = np.zeros((B, MODES), np.float64)
        x1_64 = x1.astype(np.float64)
        for bb, ff in zip(bad_b.tolist(), bad_f.tolist()):
            sdev = np.float64(s2q[bb, ff])
            delta = _retanh(s2[bb, ff], w2) - sdev * x1_64   # (64,)
            dh[bb, :] += lin1_w[:, 64 * ff : 64 * (ff + 1)].astype(np.float64) @ delta
        dh = (dh * SCALE).astype(np.float32)       # pre-scaled like the psum

    # ---- fold X1 into lin1_w, quantize fp8 with error feedback ----
    w1p = (
        lin1_w.reshape(MODES, MODES, L).astype(np.float32)
        * x1[None, None, :].astype(np.float32)
    ) * np.float32(SCALE)                          # (o, f, l)
    w1q = _quantize_feedback(w1p)                  # (o, f, l) fp8
    del w1p

    # s2t layout: [128 p, 8 B, 512 b]; tile B row p holds f = 128B + 32*(p%4) + p//4
    # (that is where the [32,500]->[128,125] reshape DMA puts A[f, :])
    p_idx = np.arange(128)
    f_of_p = 32 * (p_idx % 4) + p_idx // 4          # within-tile f offset
    s2f = np.zeros((FPAD, B), np.float32)
    s2f[:MODES] = s2q.T
    s2t = np.zeros((128, 8, B), fp8)
    for t in range(8):
        s2t[:, t, :] = s2f[128 * t + f_of_p].astype(fp8)
    s2t = np.ascontiguousarray(s2t.reshape(128, 8 * B))

    # ones block-diagonal stationary operand: ones8[p, i, m] = (m == 16i + p//8)
    ones8 = np.zeros((128, 2, 32), np.float32)
    for p in range(128):
        for i in range(2):
            ones8[p, i, 16 * i + p // 8] = 1.0
    ones8 = ones8.astype(fp8)

    in_maps = []
    for c in range(NCORES):
        j0, j1 = JSH * c, JSH * (c + 1)
        # w1x[p, B, i, (P, c4, j)]:
        #   f = 128B + 32c4 + 16i + p//8,  l = 8P + p%8
        qp = np.zeros((JSH, FPAD, L), fp8)
        qp[:, :MODES, :] = w1q[j0:j1]
        w1x = np.ascontiguousarray(
            qp.reshape(JSH, 8, 4, 2, 16, 8, 8)         # j B c4 i mh P dl
            .transpose(4, 6, 1, 3, 5, 2, 0)            # mh dl B i P c4 j
            .reshape(128, NCHUNK, 2, CHCOL)
        )
        m = {
            "w1x": w1x,
            "s2t": s2t,
            "ones8": ones8,
            "bias": np.ascontiguousarray(lin1_b[j0:j1].reshape(JSH, 1)),
            "l2t": np.ascontiguousarray(lin2_w[:, j0:j1].T),
        }
        if use_dh:
            m["dht"] = np.ascontiguousarray(dh[:, j0:j1].T)
        in_maps.append(m)
    return use_dh, in_maps


def kernel(
    params,
    wfft0_re,
    wfft0_im,
    wfft1_re,
    wfft1_im,
    wfft2_re,
    wfft2_im,
    lin1_w,
    lin1_b,
    lin2_w,
    lin2_b,
):
    from concourse.bass_utils import run_bass_kernel_spmd

    use_dh, in_maps = _host_prep(
        params, wfft0_re, wfft0_im, wfft1_re, wfft1_im, wfft2_re, wfft2_im,
        lin1_w, lin1_b, lin2_w, lin2_b,
    )
    nc = _build_program(use_dh)
    _CACHE["last_run"] = (nc, in_maps)
    res = run_bass_kernel_spmd(nc, in_maps, list(range(NCORES)))

    acc = np.zeros((L, B), np.float64)
    for c in range(NCORES):
        acc += res.results[c]["outp"].astype(np.float64)
    out = 1.0 / (1.0 + np.exp(-(acc.T + lin2_b.astype(np.float64))))
    return out.astype(np.float32)


# revision 20
# speedup vs baseline: 1.9264x; 1.0458x over previous
"""Trainium2 Bass kernel for nn_Metamorph_parameterReinforcer.

Math background (exact identities, verified against the reference):
  The reference's einsum("bfp,mn->bfm", fx, wfft) sums over BOTH p and n,
  so each "STFT block" collapses:
    sum_p fft(x, norm=forward)[..., p] == x[..., 0]
    block(x)[b, f, k] = Re tanh(x[b, f, 0] * W[k]),
       W[k] = sum_m (sum_n wfft[m, n]) * exp(2j*pi*k*m/64)
  Chaining three blocks, only element 0 of the last axis propagates:
    a  = params[:, :, 0]
    s1 = Retanh(a  * W0[0]);  s2 = Retanh(s1 * W1[0])
    x3[b, f, l] = Retanh(s2[b, f] * W2[l])         # (512, 1000, 64)
    h  = tanh(x3.reshape(512, 64000) @ lin1_w.T + lin1_b)
    out = sigmoid(h @ lin2_w.T + lin2_b)
  Because |W0[0]|, |W1[0]| ~ 32000, tanh saturates and s2 is +-1 in f32 for
  all but (rare) tiny-|a| entries; where s2 = +-1, x3[b, f, :] =
  s2[b, f] * X1[:] with X1 = Retanh(W2) -- exactly rank-1.  Rare
  non-saturated entries get an exact correction (dht) added before the lin1
  tanh.

Device kernel (8 cores, lin1_w sharded over its output dim j, 125 rows/core).
v2 design (fp8 + DoubleRow + constant stationary operand):
  * X1 is folded into lin1_w on the host: w1'[o, f, l] = lin1_w[o, 64f+l] *
    X1[l].  Stage 1 then reduces w1' over l with a CONSTANT ones
    block-diagonal stationary operand, so every matmul shares one lhsT and
    columns from different f-groups ride in the same matmul.
  * w1' is quantized to fp8e4 (TRN E4M3, max 240) with ERROR-FEEDBACK along
    l: the rounding error of element l is carried into element l+1, so the
    per-(o,f) SUM over l is accurate to ~1 ulp instead of sqrt(64) ulps.
    Simulated end-to-end rel err 2.4e-3 (vs 1.7e-2 naive fp8, 1.5e-3 bf16).
  * DoubleRow perf mode: virtual K=256 (2 fp8 per PE cell), halving both
    DMA bytes (8.4 MB/core) and PE streaming time vs bf16.  DoubleRow
    outputs must sit at psum partition 0 (walrus ISA check), so:
  * f padded 1000->1024, split into 8 banks of 128.  Bank B accumulates
    psum [32, 500] over 8 passes: psum[m, 125*c4 + j] = A[f, j] with
    f = 128B + 32*c4 + m.  One chunk = one bank's full fp8 stream (1 MB).
  * Per bank: DVE/ScalarE copy [32, 500] psum->SBUF (bf16), then ONE
    gpsimd SBUF->SBUF DMA reshapes [32, 500] -> [128, 125] (partition p =
    4m + c4; the host permutes s2t rows to match), giving the [128 f, 125 j]
    lhsT tile for stage 2.  All per-bank work overlaps later banks' stream.
  * stage 2: h[j, b] = tanh(2^-11 * sum_f A[f, j] s2[f, b] + bias) -- 8
    accumulating matmuls (K = 128 f's each) + ScalarE tanh (the activation
    scale folds the fp8 2^11 quantization scale away).
  * stage 3: partial[k, b] = sum_j l2t[j, k] h[j, b]  (one matmul)
Host combines the 8 partials: out = sigmoid(sum_c partial_c + lin2_b).
"""

import numpy as np

B, MODES, L = 512, 1000, 64
NCORES = 8
JSH = MODES // NCORES          # 125 lin1 output rows per core
FPAD = 1024                    # padded mode count (8 stage-2 K-tiles of 128)
NCHUNK = 8                     # w1x DMA chunks = psum banks (1 MB each)
CHCOL = 4000                   # columns per chunk: (P, c4, j) = 8*4*125
SCALE = 2.0 ** 11              # fp8 quantization scale for w1'
SAT = 50.0                     # |2*s*Re(W)| beyond this: Retanh == sign


def _retanh(s, w):
    """Re tanh(s * w) for real array s and complex (array or scalar) w."""
    s = np.asarray(s, np.float64)
    x = 2.0 * np.multiply.outer(s, np.real(w))
    y = 2.0 * np.multiply.outer(s, np.imag(w))
    xc = np.clip(x, -SAT, SAT)
    with np.errstate(over="ignore", invalid="ignore"):
        r = np.sinh(xc) / (np.cosh(xc) + np.cos(y))
    return np.where(np.abs(x) >= SAT, np.sign(x), r)


def _wvec(wre, wim):
    """W[k] = sum_m (sum_n w[m, n]) * exp(2j pi k m / L)."""
    wsum = wre.astype(np.float64).sum(axis=1) + 1j * wim.astype(np.float64).sum(axis=1)
    tw = np.exp(2j * np.pi * np.outer(np.arange(L), np.arange(L)) / L)
    return tw @ wsum


_CACHE = {}


def _build_program(use_dh):
    """Build (and cache) the Bass program. Same program for all 8 cores."""
    key = ("prog", use_dh, "fp8dr_v5")
    if key in _CACHE:
        return _CACHE[key]

    import concourse.bacc as bacc
    import concourse.mybir as mybir
    import concourse.tile as tile

    f32 = mybir.dt.float32
    f32r = mybir.dt.float32r
    bf16 = mybir.dt.bfloat16
    fp8 = mybir.dt.float8e4
    DR = mybir.MatmulPerfMode.DoubleRow
    nc = bacc.Bacc("TRN2", target_bir_lowering=False, debug=False)

    # [p, bank, (P, i), c4*j]: each (p, bank) is one contiguous 8 KB run in
    # DRAM, and each (p, bank, pass-half) is a contiguous 4 KB run, so both
    # full-bank and half-bank DMAs keep fat descriptors (HWDGE descriptor
    # generation is the ring-throughput limit for runs under ~4 KB).
    w1x_d = nc.dram_tensor("w1x", [128, NCHUNK, 16, 500], fp8, kind="ExternalInput")
    s2t_d = nc.dram_tensor("s2t", [128, 8 * B], fp8, kind="ExternalInput")
    ones_d = nc.dram_tensor("ones8", [128, 2, 32], fp8, kind="ExternalInput")
    bias_d = nc.dram_tensor("bias", [JSH, 1], f32, kind="ExternalInput")
    l2t_d = nc.dram_tensor("l2t", [JSH, L], f32r, kind="ExternalInput")
    if use_dh:
        dht_d = nc.dram_tensor("dht", [JSH, B], f32, kind="ExternalInput")
    outp_d = nc.dram_tensor("outp", [L, B], f32, kind="ExternalOutput")

    # Pass-range sub-DMA splits: first bank split so the PE starts earlier;
    # last bank split so only its last 4 matmuls trail the final bytes.
    SPLITS = {0: [4, 4], 7: [4, 4]}

    with tile.TileContext(nc) as tc:
        with (
            tc.tile_pool(name="const", bufs=1) as const,
            tc.tile_pool(name="w1pool", bufs=3) as w1pool,
            tc.tile_pool(name="s4pool", bufs=2) as s4pool,
            tc.tile_pool(name="acc", bufs=1) as acc,
            tc.tile_pool(name="psA", bufs=3, space="PSUM") as psA,
            tc.tile_pool(name="psH", bufs=1, space="PSUM") as psH,
            tc.tile_pool(name="psO", bufs=1, space="PSUM") as psO,
        ):
            # ones8 first on the sync ring (needed by the first LDWEIGHTS);
            # all other consts are issued AFTER the w1x chunks (they are only
            # needed by stage 2/3).  s2t rides the SWDGE queue.
            ones8 = const.tile([128, 2, 32], fp8)
            nc.sync.dma_start(ones8[:], ones_d.ap())
            s2t = const.tile([128, 8 * B], fp8)
            nc.gpsimd.dma_start(s2t[:], s2t_d.ap())

            at_sb = acc.tile([128, 8 * JSH], bf16)
            ph = psH.tile([JSH, B], f32)

            dma_engines = [nc.sync, nc.scalar]

            for bk in range(NCHUNK):
                eng = dma_engines[bk % 2]
                w1c = w1pool.tile([128, 16, 500], fp8, tag="w1c")
                r0 = 0
                for passes in SPLITS.get(bk, [8]):
                    rw = 2 * passes
                    eng.dma_start(
                        w1c[:, r0 : r0 + rw, :],
                        w1x_d.ap()[:, bk, r0 : r0 + rw, :],
                    )
                    r0 += rw
                pa = psA.tile([32, 512], f32, name="pa", tag="pa")
                for p8 in range(8):
                    nc.tensor.matmul(
                        pa[:, 0:500],
                        ones8[:],
                        w1c[:, 2 * p8 : 2 * p8 + 2, :],
                        start=(p8 == 0),
                        stop=(p8 == 7),
                        perf_mode=DR,
                    )
                # psum [32, 500] -> SBUF bf16, then reshape-DMA to the
                # [128, 125] stage-2 lhsT tile (partition p = 4m + c4)
                s4 = s4pool.tile([32, 500], bf16, tag="s4")
                if bk % 2 == 0:
                    nc.vector.tensor_copy(s4[:], pa[:, 0:500])
                else:
                    nc.scalar.activation(
                        s4[:], pa[:, 0:500], mybir.ActivationFunctionType.Copy
                    )
                nc.gpsimd.dma_start(at_sb[:, JSH * bk : JSH * (bk + 1)], s4[:])
                nc.tensor.matmul(
                    ph[:, :],
                    at_sb[:, JSH * bk : JSH * (bk + 1)],
                    s2t[:, B * bk : B * (bk + 1)],
                    start=(bk == 0),
                    stop=(bk == 7),
                )

            # late consts (needed only for the activation / stage 3)
            bias = const.tile([JSH, 1], f32)
            nc.sync.dma_start(bias[:], bias_d.ap())
            l2t = const.tile([JSH, L], f32r)
            nc.sync.dma_start(l2t[:], l2t_d.ap())
            if use_dh:
                dht = const.tile([JSH, B], f32)
                nc.sync.dma_start(dht[:], dht_d.ap())
                nc.vector.tensor_add(ph[:, :], ph[:, :], dht[:, :])

            # ---- tanh + stage 3 + output, pipelined in two column halves
            # (separate psum banks so the zero regions stay independent) ----
            h_sb = acc.tile([JSH, B], f32r)
            o_sb = acc.tile([L, B], f32)
            H = B // 2
            for hf in range(2):
                cs = slice(H * hf, H * (hf + 1))
                nc.scalar.activation(
                    h_sb[:, cs],
                    ph[:, cs],
                    mybir.ActivationFunctionType.Tanh,
                    bias=bias[:, 0:1],
                    scale=float(1.0 / SCALE),
                )
                po = psO.tile([L, H], f32, name=f"po{hf}", tag=f"po{hf}")
                nc.tensor.matmul(
                    po[:, :], l2t[:, :], h_sb[:, cs], start=True, stop=True
                )
                nc.vector.tensor_copy(o_sb[:, cs], po[:, :])
                nc.sync.dma_start(outp_d.ap()[:, cs], o_sb[:, cs])

    nc.compile()
    _CACHE[key] = nc
    return nc


def _quantize_feedback(w1p):
    """fp8e4 (TRN E4M3) quantization with error feedback along the last axis.

    Carrying each element's rounding error into the next keeps the sum over
    the last axis accurate to ~1 ulp of a single element.
    """
    import ml_dtypes

    fp8 = ml_dtypes.float8_e4m3
    q = np.empty(w1p.shape, dtype=fp8)
    err = np.zeros(w1p.shape[:-1], np.float32)
    for l in range(w1p.shape[-1]):
        v = w1p[..., l] + err
        ql = np.clip(v, -240, 240).astype(fp8)
        q[..., l] = ql
        err = v - ql.astype(np.float32)
    return q


def profile_last(trace_cores=None):
    """Re-run the last-built program with NTFF tracing (dev/test helper)."""
    if "last_run" not in _CACHE:
        return None
    from concourse.bass_utils import run_bass_kernel_spmd

    nc, in_maps = _CACHE["last_run"]
    return run_bass_kernel_spmd(
        nc,
        in_maps,
        list(range(NCORES)),
        trace=True,
        trace_cores=trace_cores,
    )


def _host_prep(
    params,
    wfft0_re,
    wfft0_im,
    wfft1_re,
    wfft1_im,
    wfft2_re,
    wfft2_im,
    lin1_w,
    lin1_b,
    lin2_w,
    lin2_b,
):
    """All host-side prep: collapse, quantize, per-core shards."""
    import ml_dtypes

    fp8 = ml_dtypes.float8_e4m3
    bf16 = ml_dtypes.bfloat16

    a = params[:, :, 0].astype(np.float64)
    w0 = _wvec(wfft0_re, wfft0_im)[0]
    w1v = _wvec(wfft1_re, wfft1_im)[0]
    w2 = _wvec(wfft2_re, wfft2_im)
    s1 = _retanh(a, w0)
    s2 = _retanh(s1, w1v)                          # (B, MODES) f64
    x1 = _retanh(np.float64(1.0), w2)              # (64,) f64

    # s2 as the device will see it (fp8 in the stage-2 rhs; +-1 is exact)
    s2q = np.clip(s2, -240, 240).astype(fp8).astype(np.float32)

    # exact correction for entries where tanh did not saturate to +-1
    bad_b, bad_f = np.nonzero(np.abs(s2q) != np.float32(1.0))
    use_dh = bad_b.size > 0
    dh = None
    if use_dh:
        dh = np.zeros((B, MODES), np.float64)
        x1_64 = x1.astype(np.float64)
        for bb, ff in zip(bad_b.tolist(), bad_f.tolist()):
            sdev = np.float64(s2q[bb, ff])
            delta = _retanh(s2[bb, ff], w2) - sdev * x1_64   # (64,)
            dh[bb, :] += lin1_w[:, 64 * ff : 64 * (ff + 1)].astype(np.float64) @ delta
        dh = (dh * SCALE).astype(np.float32)       # pre-scaled like the psum

    # ---- fold X1 into lin1_w, quantize fp8 with error feedback ----
    w1p = (
        lin1_w.reshape(MODES, MODES, L).astype(np.float32)
        * x1[None, None, :].astype(np.float32)
    ) * np.float32(SCALE)                          # (o, f, l)
    w1q = _quantize_feedback(w1p)                  # (o, f, l) fp8
    del w1p

    # s2t layout: [128 p, 8 B, 512 b]; tile B row p holds f = 128B + 32*(p%4) + p//4
    # (that is where the [32,500]->[128,125] reshape DMA puts A[f, :])
    p_idx = np.arange(128)
    f_of_p = 32 * (p_idx % 4) + p_idx // 4          # within-tile f offset
    s2f = np.zeros((FPAD, B), np.float32)
    s2f[:MODES] = s2q.T
    s2t = np.zeros((128, 8, B), fp8)
    for t in range(8):
        s2t[:, t, :] = s2f[128 * t + f_of_p].astype(fp8)
    s2t = np.ascontiguousarray(s2t.reshape(128, 8 * B))

    # ones block-diagonal stationary operand: ones8[p, i, m] = (m == 16i + p//8)
    ones8 = np.zeros((128, 2, 32), np.float32)
    for p in range(128):
        for i in range(2):
            ones8[p, i, 16 * i + p // 8] = 1.0
    ones8 = ones8.astype(fp8)

    in_maps = []
    for c in range(NCORES):
        j0, j1 = JSH * c, JSH * (c + 1)
        # w1x[p, B, (P, i), (c4, j)]:
        #   f = 128B + 32c4 + 16i + p//8,  l = 8P + p%8
        qp = np.zeros((JSH, FPAD, L), fp8)
        qp[:, :MODES, :] = w1q[j0:j1]
        w1x = np.ascontiguousarray(
            qp.reshape(JSH, 8, 4, 2, 16, 8, 8)         # j B c4 i mh P dl
            .transpose(4, 6, 1, 5, 3, 2, 0)            # mh dl B P i c4 j
            .reshape(128, NCHUNK, 16, 500)
        )
        m = {
            "w1x": w1x,
            "s2t": s2t,
            "ones8": ones8,
            "bias": np.ascontiguousarray(lin1_b[j0:j1].reshape(JSH, 1)),
            "l2t": np.ascontiguousarray(lin2_w[:, j0:j1].T),
        }
        if use_dh:
            m["dht"] = np.ascontiguousarray(dh[:, j0:j1].T)
        in_maps.append(m)
    return use_dh, in_maps


def kernel(
    params,
    wfft0_re,
    wfft0_im,
    wfft1_re,
    wfft1_im,
    wfft2_re,
    wfft2_im,
    lin1_w,
    lin1_b,
    lin2_w,
    lin2_b,
):
    from concourse.bass_utils import run_bass_kernel_spmd

    use_dh, in_maps = _host_prep(
        params, wfft0_re, wfft0_im, wfft1_re, wfft1_im, wfft2_re, wfft2_im,
        lin1_w, lin1_b, lin2_w, lin2_b,
    )
    nc = _build_program(use_dh)
    _CACHE["last_run"] = (nc, in_maps)
    res = run_bass_kernel_spmd(nc, in_maps, list(range(NCORES)))

    acc = np.zeros((L, B), np.float64)
    for c in range(NCORES):
        acc += res.results[c]["outp"].astype(np.float64)
    out = 1.0 / (1.0 + np.exp(-(acc.T + lin2_b.astype(np.float64))))
    return out.astype(np.float32)
